# revision 31
# baseline (speedup 1.0000x reference)
"""Trainium2 Bass kernel for nn_BuildCostVolume (stereo cost volume + Mamba scan).

Sharding: disparity axis (24) split as 3 per core across 8 cores.

Per-core algorithm (core k handles disparities d = 3k+j, j in 0..2):
  - Host pre-shifts featuresR right by 3k (zero-filled); in-kernel access
    patterns add the per-j shift (j in {0,1,2} is compile-time, SPMD-safe).
  - Features and projection weights live in bf16; both feature planes are
    resident in SBUF (loaded once).
  - Projections u/dt/B/C/Dterm are computed from L and shifted-R features with
    even/odd split weights (channel interleave trick) on PE.
  - dt = softplus via Exp + Ln(x+1) on ACT; decay a = exp(A[e,s]*dt) via ACT
    per-partition scale in an (s-pair x e) = 128-partition layout.
  - Mamba recurrence h = a*h + b: b = dtu*B on the Pool engine (gpsimd TT
    mult), the scan itself on DVE (tensor_tensor_scan is DVE-only).
  - y/cost contraction via PE: block-diag W_out fold, C multiply at PSUM
    eviction, partition-sum + precomputed D-term matmul.
  - Channel-attn max pool via an h-folding tree of 2x TT-max ops.
  - Emission is software-pipelined: chunk front-ends are emitted 2 chunks
    ahead so FIFO engine queues don't serialize; the per-j epilogue
    (channel MLP + spatial gating + output DMA) is emitted as soon as a
    j's two chunks are done, overlapping later chunks.
"""
import os
import numpy as np

C, H, W, DV = 32, 64, 128, 24
_ITERS = int(os.environ.get("KERNEL_ITERS", "1"))
E, S, G = 64, 8, 8
NCORES, JD = 8, 3          # cores, disparities per core
PAD = 8                    # leading zero columns in feature tensors
HH = 32                    # h rows per chunk
NCH = 6                    # chunks = (j, h-half)
CCOLS = HH * W             # 4096 columns per chunk
HW = H * W                 # 8192

_compiled = {}


def _f32(x):
    return np.ascontiguousarray(np.asarray(x, np.float32))


def _build_program():
    import concourse.bacc as bacc
    import concourse.mybir as mybir
    from concourse.tile import TileContext

    F32 = mybir.dt.float32
    BF16 = mybir.dt.bfloat16
    AF = mybir.ActivationFunctionType
    AX = mybir.AxisListType
    OP = mybir.AluOpType

    nc = bacc.Bacc("TRN2", target_bir_lowering=False, debug=False,
                   num_devices=NCORES)

    feat_d = nc.dram_tensor("feat", [C, 2 * (PAD + HW)], BF16, kind="ExternalInput").ap()
    wse_d = nc.dram_tensor("wse", [2 * C, 576], BF16, kind="ExternalInput").ap()
    wbf_d = nc.dram_tensor("wbf", [128, 48], BF16, kind="ExternalInput").ap()
    avec_d = nc.dram_tensor("avec", [128, 8], F32, kind="ExternalInput").ap()
    umask_d = nc.dram_tensor("umask", [32, JD * W], BF16, kind="ExternalInput").ap()
    mneg_d = nc.dram_tensor("mneg", [G, JD * W], BF16, kind="ExternalInput").ap()
    invc_d = nc.dram_tensor("invc", [G, JD], F32, kind="ExternalInput").ap()
    wsp_d = nc.dram_tensor("wsp", [128, 4], F32, kind="ExternalInput").ap()
    mlp_d = nc.dram_tensor("mlp", [G, 24], F32, kind="ExternalInput").ap()
    out_d = nc.dram_tensor("out", [JD * H, G, W], BF16, kind="ExternalOutput").ap()

    with TileContext(nc) as tc:
        with tc.tile_pool(name="const", bufs=1) as cpool, \
             tc.tile_pool(name="dtmp", bufs=1) as dtmpp, \
             tc.tile_pool(name="dt2", bufs=2) as dt2p, \
             tc.tile_pool(name="dtu2", bufs=2) as dtu2p, \
             tc.tile_pool(name="bc", bufs=2) as bcp, \
             tc.tile_pool(name="bb", bufs=2) as bbp, \
             tc.tile_pool(name="csm", bufs=1) as csmp, \
             tc.tile_pool(name="apool", bufs=1) as apl, \
             tc.tile_pool(name="bpool", bufs=2) as bpl, \
             tc.tile_pool(name="hpool", bufs=4) as hpl, \
             tc.tile_pool(name="tpool", bufs=4) as tpl, \
             tc.tile_pool(name="cstg", bufs=1) as cstgp, \
             tc.tile_pool(name="epi", bufs=1) as epi, \
             tc.tile_pool(name="pproj", bufs=2, space="PSUM") as pproj, \
             tc.tile_pool(name="pz", bufs=1, space="PSUM") as pz, \
             tc.tile_pool(name="pc", bufs=1, space="PSUM") as pc:

            _ld = mybir.InstLoadActFuncSet(
                name=nc.get_next_instruction_name(), act_func_set_id=6,
                ins=[], outs=[])
            nc.scalar.add_instruction(_ld)
            # consts, ordered by first use; spread across queues so the
            # first chunk's front-end can start ASAP.
            wseL = cpool.tile([C, 576], BF16)
            nc.scalar.dma_start(wseL[:], wse_d[0:C, :])
            wseR = cpool.tile([C, 576], BF16)
            nc.scalar.dma_start(wseR[:], wse_d[C:2 * C, :])
            avec = cpool.tile([128, 8], F32)
            nc.scalar.dma_start(avec[:], avec_d[:])
            umask = cpool.tile([32, JD * W], BF16)
            nc.scalar.dma_start(umask[:], umask_d[:])
            wbf = cpool.tile([128, 48], BF16)
            nc.gpsimd.dma_start(wbf[:], wbf_d[:])
            # resident feature planes: L and R halves of feat_d, quarter by
            # quarter in consumption order (chunk 0 uses hh=0 first).
            featL = cpool.tile([C, PAD + HW], BF16)
            featR = cpool.tile([C, PAD + HW], BF16)
            nc.gpsimd.dma_start(featL[:, 0:PAD], feat_d[:, 0:PAD])
            nc.gpsimd.dma_start(featR[:, 0:PAD], feat_d[:, PAD + HW: 2 * PAD + HW])
            for half in range(2):
                hb = half * CCOLS
                for q in range(2):
                    s = hb + q * 2048
                    nc.sync.dma_start(featL[:, PAD + s: PAD + s + 2048],
                                      feat_d[:, PAD + s: PAD + s + 2048])
                    nc.sync.dma_start(featR[:, PAD + s: PAD + s + 2048],
                                      feat_d[:, 2 * PAD + HW + s: 2 * PAD + HW + s + 2048])
            mneg = cpool.tile([G, JD * W], BF16)
            nc.gpsimd.dma_start(mneg[:], mneg_d[:])
            invc = cpool.tile([G, JD], F32)
            nc.gpsimd.dma_start(invc[:], invc_d[:])
            wsp = cpool.tile([128, 4], F32)
            nc.gpsimd.dma_start(wsp[:], wsp_d[:])
            mlpw = cpool.tile([G, 24], F32)
            nc.gpsimd.dma_start(mlpw[:], mlp_d[:])

            def _one_iter(it):
              tg = f"i{it}"
              acc24 = epi.tile([G, 24], F32, tag="acc24")    # per-(chunk,s4) sums
              mx6 = epi.tile([G, 8], F32, tag="mx6")         # per-chunk maxes
              S0 = epi.tile([64, G * W], BF16, tag="S0")
              S1 = epi.tile([64, G * W], BF16, tag="S1")
              S2 = epi.tile([64, G * W], BF16, tag="S2")
              Sj = [S0, S1, S2]

              state = {}

              def phase_A_alloc(ch):
                  """Allocate the front-end tiles for one chunk."""
                  dt2 = dt2p.tile([128, CCOLS], BF16)
                  dtu2 = dtu2p.tile([128, CCOLS], BF16)
                  bc = bcp.tile([32, CCOLS], BF16)
                  state[ch] = (dt2, dtu2, bc)

              def phase_A_s4(ch, s4):
                  """Projections + PSUM evictions for one quarter of a chunk."""
                  j, hh = divmod(ch, 2)
                  base = hh * CCOLS
                  dt2, dtu2, bc = state[ch]
                  if True:
                      cs = base + s4 * 1024
                      sl = slice(s4 * 1024, s4 * 1024 + 1024)
                      ftL = featL[:, PAD + cs: PAD + cs + 1024]
                      ftR = featR[:, PAD + cs - j: PAD + cs + 1024 - j]

                      # dt stream (L-hv0, L-hv1, R-hv0, R-hv1: 2 Ldweights)
                      pd = pproj.tile([128, 1024], F32, tag="proj")
                      for hv in range(2):
                          cv = slice(512 * hv, 512 * hv + 512)
                          nc.tensor.matmul(pd[:, cv], lhsT=wseL[:, 256:384],
                                           rhs=ftL[:, cv], start=True, stop=False)
                      for hv in range(2):
                          cv = slice(512 * hv, 512 * hv + 512)
                          nc.tensor.matmul(pd[:, cv], lhsT=wseR[:, 384:512],
                                           rhs=ftR[:, cv], start=False, stop=True)
                      dm = dtmpp.tile([128, 1024], BF16)
                      nc.scalar.activation(dm[:], pd[:], AF.Exp,
                                           bias=avec[:, 0:1], scale=1.0)
                      nc.scalar.activation(dt2[:, sl], dm[:], AF.Ln, bias=1.0,
                                           scale=1.0)

                      # u stream -> dtu = dt * u (PSUM operand TT)
                      pu = pproj.tile([128, 1024], F32, tag="proj")
                      for hv in range(2):
                          cv = slice(512 * hv, 512 * hv + 512)
                          nc.tensor.matmul(pu[:, cv], lhsT=wseL[:, 0:128],
                                           rhs=ftL[:, cv], start=True, stop=False)
                      for hv in range(2):
                          cv = slice(512 * hv, 512 * hv + 512)
                          nc.tensor.matmul(pu[:, cv], lhsT=wseR[:, 128:256],
                                           rhs=ftR[:, cv], start=False, stop=True)
                      nc.vector.tensor_tensor(dtu2[:, sl], dt2[:, sl], pu[:],
                                              OP.mult)

                      # D|B|C stream, masked at eviction
                      pb = pproj.tile([128, 1024], F32, tag="proj")
                      for hv in range(2):
                          cv = slice(512 * hv, 512 * hv + 512)
                          nc.tensor.matmul(pb[0:32, cv], lhsT=wseL[:, 512:544],
                                           rhs=ftL[:, cv], start=True, stop=False)
                      for hv in range(2):
                          cv = slice(512 * hv, 512 * hv + 512)
                          nc.tensor.matmul(pb[0:32, cv], lhsT=wseR[:, 544:576],
                                           rhs=ftR[:, cv], start=False, stop=True)
                      mview = umask[:, j * W:(j + 1) * W].unsqueeze(1) \
                          .broadcast_to((32, 8, W))
                      nc.vector.scalar_tensor_tensor(
                          bc[:, sl].rearrange("p (a b) -> p a b", b=W),
                          pb[0:32, :].rearrange("p (a b) -> p a b", b=W), 1.0,
                          mview, OP.mult, OP.mult)

              def emit_bb(ch, p):
                  dt2, dtu2, bc = state[ch][:3]
                  bb = bbp.tile([128, CCOLS], BF16)
                  nc.sync.dma_start(
                      bb[:],
                      bc[8 + 2 * p:8 + 2 * p + 2, :].unsqueeze(1)
                      .broadcast_to((2, 64, CCOLS)))
                  return bb

              def emit_av(ch, p):
                  dt2 = state[ch][0]
                  av = apl.tile([128, CCOLS], BF16, tag=f"av{p & 1}")
                  nc.scalar.activation(
                      av[:].rearrange("p (h w) -> p h w", w=W)[:, :, 1:W],
                      dt2[:].rearrange("p (h w) -> p h w", w=W)[:, :, 1:W],
                      AF.Exp, bias=0.0, scale=avec[:, 1 + p: 2 + p])
                  return av

              def emit_scan(ch, p, bb, av, htiles):
                  dt2, dtu2, bc = state[ch][:3]
                  bt = bpl.tile([128, CCOLS], BF16)
                  nc.gpsimd.tensor_tensor(bt[:], dtu2[:], bb[:], OP.mult)
                  hT = hpl.tile([128, CCOLS], BF16)
                  nc.vector.tensor_tensor_scan(hT[:], av[:], bt[:], 0.0,
                                               OP.mult, OP.add)
                  htiles.append(hT)

              def phase_C(ch, htiles):
                  """W_out fold, C gate, cost, pools, S staging."""
                  j, hh = divmod(ch, 2)
                  dt2, dtu2, bc = state[ch][:3]
                  csm = csmp.tile([128, CCOLS], BF16)
                  nc.sync.dma_start(
                      csm[:],
                      bc[16:24, :].unsqueeze(1)
                      .broadcast_to((8, 16, CCOLS)))

                  tts = []
                  for s8 in range(4):
                      sl10 = slice(s8 * 1024, s8 * 1024 + 1024)
                      zp = pz.tile([128, 1024], F32, tag="zp")
                      for half in range(2):
                          zv = slice(512 * half, 512 * half + 512)
                          sl5 = slice(s8 * 1024 + 512 * half,
                                      s8 * 1024 + 512 * half + 512)
                          for p in range(4):
                              nc.tensor.matmul(zp[32 * p:32 * p + 32, zv],
                                               lhsT=wbf[:, 0:32],
                                               rhs=htiles[p][:, sl5],
                                               start=True, stop=True,
                                               tile_position=(0, 32 * p))
                      tt = tpl.tile([128, 1024], BF16)
                      nc.vector.scalar_tensor_tensor(tt[:], zp[:], 1.0,
                                                     csm[:, sl10], OP.mult, OP.mult)
                      tts.append(tt)

                  cstg = cstgp.tile([8, CCOLS], BF16)
                  for s4 in range(4):
                      sl = slice(s4 * 1024, s4 * 1024 + 1024)
                      cp = pc.tile([8, 1024], F32, tag="cp")
                      for hv in range(2):
                          cv = slice(512 * hv, 512 * hv + 512)
                          cg = slice(s4 * 1024 + 512 * hv, s4 * 1024 + 512 * hv + 512)
                          nc.tensor.matmul(cp[:, cv], lhsT=wbf[:, 32:40],
                                           rhs=tts[s4][:, cv], start=True, stop=False)
                          nc.tensor.matmul(cp[:, cv], lhsT=wbf[0:8, 40:48],
                                           rhs=bc[0:8, cg], start=False, stop=True)
                      nc.scalar.activation(
                          cstg[:, sl], cp[:], AF.Copy, bias=0.0, scale=1.0,
                          accum_out=acc24[:, ch * 4 + s4: ch * 4 + s4 + 1])

                  # masked per-chunk max: tree-max over h (2x TT ops keep the
                  # w structure), then mask+reduce on the final [G, W] row-max
                  mscr = epi.tile([G, 1920], BF16, tag="mscr")
                  nc.vector.tensor_tensor(mscr[:, 0:1024], cstg[:, 0:1024],
                                          cstg[:, 1024:2048], OP.max)
                  nc.vector.tensor_tensor(mscr[:, 0:1024], mscr[:, 0:1024],
                                          cstg[:, 2048:3072], OP.max)
                  nc.vector.tensor_tensor(mscr[:, 0:1024], mscr[:, 0:1024],
                                          cstg[:, 3072:4096], OP.max)
                  nc.vector.tensor_tensor(mscr[:, 1024:1536],
                                          mscr[:, 0:512],
                                          mscr[:, 512:1024], OP.max)
                  nc.vector.tensor_tensor(mscr[:, 1536:1792],
                                          mscr[:, 1024:1280],
                                          mscr[:, 1280:1536], OP.max)
                  nc.vector.tensor_tensor(mscr[:, 1792:1920],
                                          mscr[:, 1536:1664],
                                          mscr[:, 1664:1792], OP.max)
                  mwv = epi.tile([G, W], F32, tag="mwv")
                  nc.vector.tensor_tensor(mwv[:], mscr[:, 1792:1920],
                                          mneg[:, j * W:(j + 1) * W], OP.add)
                  nc.vector.tensor_reduce(mx6[:, ch:ch + 1],
                                          mwv[:].unsqueeze(1), AX.X, OP.max)
                  # stage into S_j rows hh*32..hh*32+32, layout [h, (g, w)]
                  for g in range(G):
                      nc.scalar.dma_start(
                          Sj[j][hh * 32:hh * 32 + 32, g * W:(g + 1) * W],
                          cstg[g:g + 1, :].rearrange("p (h w) -> p h w", w=W))

              def phase_E(j):
                  """Per-j epilogue: channel MLP gate + spatial attention."""
                  avg = epi.tile([G, 1], F32, tag="avg")
                  nc.vector.tensor_reduce(
                      avg[:], acc24[:, 8 * j: 8 * j + 8].unsqueeze(1), AX.X, OP.add)
                  nc.vector.tensor_tensor(avg[:], avg[:], invc[:, j:j + 1], OP.mult)
                  mx = epi.tile([G, 1], F32, tag="mx")
                  nc.vector.tensor_reduce(
                      mx[:], mx6[:, 2 * j:2 * j + 2].unsqueeze(1), AX.X, OP.max)

                  ppool = epi.tile([G, 2], F32, tag="ppool")
                  nc.vector.tensor_copy(ppool[:, 0:1], avg[:])
                  nc.vector.tensor_copy(ppool[:, 1:2], mx[:])

                  z1p = pc.tile([4, 2], F32, tag="cp")
                  nc.tensor.matmul(z1p[:], lhsT=mlpw[:, 0:4], rhs=ppool[:],
                                   start=True, stop=True)
                  z1 = epi.tile([5, 2], F32, tag="z1")
                  nc.scalar.activation(z1[0:4, :], z1p[:], AF.Relu,
                                       bias=mlpw[0:4, 12:13], scale=1.0)
                  nc.scalar.dma_start(z1[4:5, 0:2], mlpw[0:1, 16:18])
                  gp = pc.tile([1, 8], F32, tag="cp")
                  nc.tensor.matmul(gp[:], lhsT=z1[:, 0:1], rhs=mlpw[0:5, 4:12],
                                   start=True, stop=False)
                  nc.tensor.matmul(gp[:], lhsT=z1[:, 1:2], rhs=mlpw[0:5, 4:12],
                                   start=False, stop=True)
                  # sigmoid via exp(-x) + reciprocal (keeps act set 6 loaded)
                  eg = epi.tile([1, 8], F32, tag="eg")
                  nc.scalar.activation(eg[:], gp[:], AF.Exp, bias=0.0,
                                       scale=-1.0)
                  nc.vector.tensor_scalar_add(eg[:], eg[:], 1.0)
                  chg = epi.tile([1, 8], BF16, tag="chg")
                  with nc.allow_low_precision(reason="sigmoid gate in bf16"):
                      nc.vector.reciprocal(chg[:], eg[:])
                  gb = epi.tile([64, 8], BF16, tag="gb")
                  nc.scalar.dma_start(
                      gb[:], chg[0:1, :].unsqueeze(1).broadcast_to((1, 64, 8)))

                  Sg = Sj[j]
                  Sgf = epi.tile([64, G * W], BF16, tag="sgf")
                  gview = gb[:, :].unsqueeze(2).broadcast_to((64, G, W))
                  nc.vector.tensor_tensor(
                      Sgf[:].rearrange("p (a b) -> p a b", b=W),
                      Sg[:].rearrange("p (a b) -> p a b", b=W), gview, OP.mult)
                  sv = Sgf[:].rearrange("p (g w) -> p w g", g=G)
                  ssum = epi.tile([64, W], F32, tag="ss")
                  nc.vector.tensor_reduce(ssum[:], sv, AX.X, OP.add)
                  smx = epi.tile([64, W], F32, tag="sm")
                  nc.vector.tensor_reduce(smx[:], sv, AX.X, OP.max)
                  q1 = epi.tile([64, W], F32, tag="q1")
                  nc.vector.tensor_scalar_mul(q1[:], smx[:], wsp[0:64, 1:2])
                  gi = epi.tile([64, W], F32, tag="gi")
                  nc.vector.scalar_tensor_tensor(gi[:], ssum[:], wsp[0:64, 0:1],
                                                 q1[:], OP.mult, OP.add)
                  esg = epi.tile([64, W], F32, tag="esg")
                  nc.scalar.activation(esg[:], gi[:], AF.Exp,
                                       bias=wsp[0:64, 3:4], scale=-1.0)
                  nc.vector.tensor_scalar_add(esg[:], esg[:], 1.0)
                  sg = epi.tile([64, W], BF16, tag="sgate")
                  with nc.allow_low_precision(reason="sigmoid gate in bf16"):
                      nc.vector.reciprocal(sg[:], esg[:])
                  O = epi.tile([64, G * W], BF16, tag="scr4k")
                  oview = sg[:].unsqueeze(1).broadcast_to((64, G, W))
                  nc.vector.tensor_tensor(
                      O[:].rearrange("p (a b) -> p a b", b=W),
                      Sgf[:].rearrange("p (a b) -> p a b", b=W), oview, OP.mult)
                  nc.scalar.dma_start(
                      out_d[64 * j:64 * j + 64].rearrange("r g w -> r (g w)"), O[:])

              # zero the scan-reset column (w=0) of the two rotating av
              # buffers once; Exp writes only cols 1..W-1 afterwards.
              av0 = apl.tile([128, CCOLS], BF16, tag="av0")
              av1 = apl.tile([128, CCOLS], BF16, tag="av1")
              for avi in (av0, av1):
                  nc.vector.memset(
                      avi[:].rearrange("p (h w) -> p h w", w=W)[:, :, 0:1], 0)

              # chunk 0 front-end up front, then per-pair interleave: the
              # next chunk's s4 front-end groups are woven between this
              # chunk's (bt, scan) pair groups so every engine FIFO stays
              # primed without head-of-line blocking.
              phase_A_alloc(0)
              for s4 in range(4):
                  phase_A_s4(0, s4)
              for ch in range(NCH):
                  bbs = [emit_bb(ch, p) for p in range(4)]
                  avs = [emit_av(ch, 0), emit_av(ch, 1), None, None]
                  if ch + 1 < NCH:
                      phase_A_alloc(ch + 1)
                  htiles = []
                  for p in range(4):
                      if ch + 1 < NCH:
                          phase_A_s4(ch + 1, p)
                      emit_scan(ch, p, bbs[p], avs[p], htiles)
                      if p + 2 < 4:
                          avs[p + 2] = emit_av(ch, p + 2)
                  phase_C(ch, htiles)
                  del state[ch]
                  if ch % 2 == 1:
                      phase_E(ch // 2)

            for _it in range(_ITERS):
                _one_iter(_it)

    nc.compile()
    return nc


def _host_inputs(inputs):
    """Build the 8 per-core input maps from the full problem inputs."""
    L = _f32(inputs["featuresL"])[0]          # [C,H,W]
    R = _f32(inputs["featuresR"])[0]
    W_in = _f32(inputs["W_in"])
    W_dt = _f32(inputs["W_dt"])
    b_dt = _f32(inputs["b_dt"])
    W_B = _f32(inputs["W_B"])
    W_C = _f32(inputs["W_C"])
    A = -np.exp(_f32(inputs["A_log"]))        # [E,S]
    D_skip = _f32(inputs["D_skip"])
    W_out = _f32(inputs["W_out"])
    W1, b1 = _f32(inputs["W1"]), _f32(inputs["b1"])
    W2, b2 = _f32(inputs["W2"]), _f32(inputs["b2"])
    w_sp, b_sp = _f32(inputs["w_sp"]), _f32(inputs["b_sp"])

    # stationary weights [64, 576]
    idx = np.arange(128) % 64
    wse = np.zeros((2 * C, 576), np.float32)
    wse[0:32, 0:128] = W_in[0::2][:, idx]
    wse[32:64, 128:256] = W_in[1::2][:, idx]
    wse[0:32, 256:384] = W_dt[0::2][:, idx]
    wse[32:64, 384:512] = W_dt[1::2][:, idx]
    W_comb = W_in @ (D_skip[:, None] * W_out)        # [64(c), G]
    wse[0:32, 512:520] = W_comb[0::2]
    wse[0:32, 520:528] = W_B[0::2]
    wse[0:32, 528:536] = W_C[0::2]
    wse[32:64, 544:552] = W_comb[1::2]
    wse[32:64, 552:560] = W_B[1::2]
    wse[32:64, 560:568] = W_C[1::2]

    # bf16 stationaries [128, 48]
    wbf = np.zeros((128, 48), np.float32)
    for row in range(128):
        cc, e = divmod(row, 64)
        for q in range(32):
            c2, g = q // 16, q % 16
            if g < 8 and cc == c2:
                wbf[row, q] = W_out[e, g]
    for p4 in range(4):
        for local in range(32):
            c2, g = local // 16, local % 16
            if g < 8:
                wbf[32 * p4 + local, 32 + g] = 1.0
    wbf[0:8, 40:48] = np.eye(8, dtype=np.float32)

    avec = np.zeros((128, 8), np.float32)
    avec[:, 0] = b_dt[idx]
    for p4 in range(4):
        cc = np.arange(128) // 64
        avec[:, 1 + p4] = A[idx, 2 * p4 + cc]

    wspv = np.zeros((128, 4), np.float32)
    wspv[:, 0] = w_sp[0] / G
    wspv[:, 1] = w_sp[1]
    wspv[:, 2] = np.float32(np.asarray(b_sp).reshape(-1)[0]) if np.asarray(b_sp).size else 0.0
    wspv[:, 3] = -wspv[:, 2]

    mlpv = np.zeros((G, 24), np.float32)
    mlpv[:, 0:4] = W1
    mlpv[0:4, 4:12] = W2
    mlpv[4, 4:12] = 2.0 * b2
    mlpv[0:4, 12] = b1
    mlpv[0, 16] = 1.0          # z1 bias row: [1, 0] for (avg, mx) cols

    maps = []
    wi = np.arange(W)
    import ml_dtypes
    for k in range(NCORES):
        d0 = JD * k
        Rsh = np.zeros_like(R)
        if d0 > 0:
            Rsh[:, :, d0:] = R[:, :, :-d0]
        else:
            Rsh = R
        feat = np.zeros((C, 2 * (PAD + HW)), np.float32)
        feat[:, PAD:PAD + HW] = L.reshape(C, HW)
        feat[:, 2 * PAD + HW:] = Rsh.reshape(C, HW)

        umask = np.zeros((32, JD * W), np.float32)
        for j in range(JD):
            umask[:, j * W:(j + 1) * W] = (wi >= d0 + j).astype(np.float32)[None]

        mneg = np.zeros((G, JD * W), np.float32)
        for j in range(JD):
            mneg[:, j * W:(j + 1) * W] = np.where(wi >= d0 + j, 0.0, -1e30)[None]

        invc = np.zeros((G, JD), np.float32)
        for j in range(JD):
            invc[:, j] = 1.0 / (H * (W - (d0 + j)))

        maps.append({
            "feat": feat.astype(ml_dtypes.bfloat16),
            "wse": wse.astype(ml_dtypes.bfloat16),
            "wbf": wbf.astype(ml_dtypes.bfloat16),
            "avec": avec,
            "umask": umask.astype(ml_dtypes.bfloat16),
            "mneg": mneg.astype(ml_dtypes.bfloat16),
            "invc": invc,
            "wsp": wspv,
            "mlp": mlpv,
        })
    return maps


def kernel(**inputs):
    from concourse.bass_utils import run_bass_kernel_spmd

    if "nc" not in _compiled:
        _compiled["nc"] = _build_program()
    nc = _compiled["nc"]

    maps = _host_inputs(inputs)
    res = run_bass_kernel_spmd(nc, maps, list(range(NCORES))).results

    vol = np.zeros((1, G, DV, H, W), np.float32)
    for k in range(NCORES):
        o = np.asarray(res[k]["out"], np.float32).reshape(JD, H, G, W)  # [j,h,g,w]
        vol[0, :, JD * k:JD * k + JD] = np.transpose(o, (2, 0, 1, 3))
    return vol
